# revision 9
# baseline (speedup 1.0000x reference)
"""CrossTeacherAttention Trainium2 kernel (fp8 DoubleRow + folded-QK).

Math per batch element b (x as [C=256, N=1024], N=H*W):
  S_t^T[m,n] = sum_c K_t[c,m] Q[c,n] with K_t = Wk Xt_t + bk, Q = Wq Xs + bq.
  Associativity folds Wk into Wq:  S_t^T = Xt_t^T @ QG,
    QG = G Xs + h,  G = Wk^T Wq (host-precomputed, x16 for fp8 range),
    h = 16 * Wk^T bq.  bk adds a per-column constant to the logits, which
  cancels exactly in the softmax over keys -> dropped.
  E_t = exp(S_t^T/256 - 4.5)  (offset keeps E in fp8e4 range; cancels in
  the O/Z ratio).  Zb_t = 3 * colsum(E_t), broadcast to 128 partitions by
  an all-3.0 stationary matmul (3 = the uniform 1/3 teacher weight:
  attn.mean(-1) of a softmax is exactly 1/N, so the cross-teacher softmax
  weights are uniform).  rec_t = 1/Zb_t.
  V_t^T from Xt_t^T @ Wv^T (bv folded into the acc init).
  out = Xs + bv + sum_t (V_t^T' E_t) * rec_t.

All matmuls are fp8e4 MatmulPerfMode.DoubleRow: one instruction
contracts 2x128=256 at 0.5 cycles/row.  fp8 operands (xs8/xt8/g8/wv8)
are quantized host-side (bit-identical to a device-side cast) so no
cast traffic or f32 teacher loads hit the device.  Engine split: Act
does only the 24 exps (the bottleneck), DVE does the PSUM drains (QG,
V) + reciprocals + normalize muls (gpsimd cannot touch PSUM on TRN2),
Pool (gpsimd) does acc init + accumulate adds + one output DMA queue.
Software pipeline: V[t] and S[t] emitted back to back, O[t] deferred
until after S[t+1] so the Act exp stream never waits on PE.

Sharding: data-parallel over batch, B=8 -> one batch element per core.
"""

import sys

sys.path.insert(0, "/opt/trn_rl_repo")

import numpy as np

import concourse.bass as bass
import concourse.tile as tile
from concourse import mybir
from concourse.bass_utils import run_bass_kernel_spmd

B, C, H, W = 8, 256, 32, 32
N = H * W  # 1024
T = 3
P = 128
CC = C // P  # 2 c-chunks
F32 = mybir.dt.float32
F8 = mybir.dt.float8e4
F8NP = mybir.dt.np(F8)
SCALE = 1.0 / 256.0  # 1/16 attention scale, 1/16 from the G x16 prescale
C0 = -4.5  # logit offset so exp() fits fp8e4 range
DR = mybir.MatmulPerfMode.DoubleRow
Exp = mybir.ActivationFunctionType.Exp


def build_nc():
    nc = bass.Bass()
    xs_d = nc.dram_tensor("xs", [CC, P, N], F32, kind="ExternalInput")
    xs8_d = nc.dram_tensor("xs8", [CC, P, N], F8, kind="ExternalInput")
    xt8_d = nc.dram_tensor("xt8", [T, CC, P, N], F8, kind="ExternalInput")
    g8_d = nc.dram_tensor("g8", [CC, P, C], F8, kind="ExternalInput")
    wv8_d = nc.dram_tensor("wv8", [CC, P, C], F8, kind="ExternalInput")
    hq_d = nc.dram_tensor("hq", [CC, P, 1], F32, kind="ExternalInput")
    bv_d = nc.dram_tensor("bv", [CC, P, 1], F32, kind="ExternalInput")
    out_d = nc.dram_tensor("out", [CC, P, N], F32, kind="ExternalOutput")

    with tile.TileContext(nc) as tc:
        with (
            tc.tile_pool(name="consts", bufs=1) as consts,
            tc.tile_pool(name="epool", bufs=8) as epool,
            tc.tile_pool(name="vpool", bufs=4) as vpool,
            tc.tile_pool(name="rpool", bufs=2) as rpool,
            tc.tile_pool(name="tpool", bufs=2) as tpool,
            tc.tile_pool(name="mm", bufs=2, space="PSUM") as mm,
            tc.tile_pool(name="zo", bufs=2, space="PSUM") as zo,
        ):
            # ---- loads, ordered by criticality (SP queue) ----
            g8 = consts.tile([P, CC, C], F8, tag="g8", name="g8")
            wv8 = consts.tile([P, CC, C], F8, tag="wv8", name="wv8")
            for j in range(CC):
                nc.sync.dma_start(out=g8[:, j, :], in_=g8_d[j])
            for j in range(CC):
                nc.sync.dma_start(out=wv8[:, j, :], in_=wv8_d[j])
            xs8 = consts.tile([P, CC, N], F8, tag="xs8", name="xs8")
            for j in range(CC):
                nc.sync.dma_start(out=xs8[:, j, :], in_=xs8_d[j])
            xt8 = [consts.tile([P, CC, N], F8, tag=f"xt8_{t}", name=f"xt8_{t}")
                   for t in range(T)]
            for j in range(CC):
                nc.sync.dma_start(out=xt8[0][:, j, :], in_=xt8_d[0, j])
            hq_sb = consts.tile([P, CC, 1], F32, tag="hq", name="hq")
            bv_sb = consts.tile([P, CC, 1], F32, tag="bv", name="bv")
            for j in range(CC):
                nc.sync.dma_start(out=hq_sb[:, j, :], in_=hq_d[j])
                nc.sync.dma_start(out=bv_sb[:, j, :], in_=bv_d[j])
            xs_sb = consts.tile([P, CC, N], F32, tag="xs", name="xs")
            for j in range(CC):
                nc.sync.dma_start(out=xs_sb[:, j, :], in_=xs_d[j])
            for t in range(1, T):
                for j in range(CC):
                    nc.sync.dma_start(out=xt8[t][:, j, :], in_=xt8_d[t, j])

            ones8 = consts.tile([P, CC, P], F8, tag="ones8", name="ones8")
            nc.gpsimd.memset(ones8, 3.0)
            c0_sb = consts.tile([P, 1], F32, tag="c0", name="c0")
            nc.gpsimd.memset(c0_sb, C0)

            # ---- QG projection (fp8 DoubleRow), drained per (co, half) ----
            qg8 = consts.tile([P, CC, N], F8, tag="qg8", name="qg8")
            qps = []
            for co in range(CC):
                qp = mm.tile([P, N], F32, tag="mm", name=f"qp{co}")
                for nq in range(4):
                    nc.tensor.matmul(
                        qp[:, nq * 256:(nq + 1) * 256],
                        g8[:, :, co * P:(co + 1) * P],
                        xs8[:, :, nq * 256:(nq + 1) * 256],
                        start=True, stop=True, perf_mode=DR,
                    )
                qps.append(qp)
            # half 0 of both co first (gates S mi=0's first matmuls)
            for hh in range(2):
                sl = slice(hh * 512, (hh + 1) * 512)
                for co in range(CC):
                    nc.vector.tensor_scalar_add(
                        qg8[:, co, sl], qps[co][:, sl], hq_sb[:, co, :])

            # ---- acc init: acc = xs + bv (Pool) ----
            acc = []
            for co in range(CC):
                a_ = consts.tile([P, N], F32, tag=f"acc{co}", name=f"acc{co}")
                nc.gpsimd.tensor_scalar_add(a_, xs_sb[:, co, :],
                                            bv_sb[:, co, :])
                acc.append(a_)

            def emit_teacher(t, e8s, v8s, recs):
                """V proj + S/exp/Zb + recip for teacher t."""
                # V^T projection -> v8 half tiles [P(m), (mi in half), c]
                v8 = [vpool.tile([P, 4, C], F8, tag="v", name=f"v{t}{h}")
                      for h in range(2)]
                for h in range(2):
                    vp = mm.tile([P, N], F32, tag="mm", name=f"vp{t}{h}")
                    for q in range(4):
                        mi = h * 4 + q
                        nc.tensor.matmul(
                            vp[:, q * 256:(q + 1) * 256],
                            xt8[t][:, :, mi * P:(mi + 1) * P],
                            wv8[:, :, :],
                            start=True, stop=True, perf_mode=DR,
                        )
                    nc.vector.tensor_copy(v8[h], vp)
                v8s.append(v8)

                # S + exp, Zb matmuls interleaved per pair
                zb = [zo.tile([P, 2, 512], F32, tag="zo", name=f"zb{t}{zt}")
                      for zt in range(2)]
                e8 = [epool.tile([P, 2, N], F8, tag="e", name=f"e{t}{p}")
                      for p in range(4)]
                for mi in range(8):
                    p, j = mi // 2, mi % 2
                    sp = mm.tile([P, N], F32, tag="mm", name=f"sp{t}{mi}")
                    for nq in range(4):
                        nc.tensor.matmul(
                            sp[:, nq * 256:(nq + 1) * 256],
                            xt8[t][:, :, mi * P:(mi + 1) * P],
                            qg8[:, :, nq * 256:(nq + 1) * 256],
                            start=True, stop=True, perf_mode=DR,
                        )
                    nc.scalar.activation(e8[p][:, j, :], sp, func=Exp,
                                         scale=SCALE, bias=c0_sb)
                    if j == 1:
                        for nq in range(4):
                            nc.tensor.matmul(
                                zb[nq // 2][:, nq % 2, 0:256],
                                ones8,
                                e8[p][:, :, nq * 256:(nq + 1) * 256],
                                start=(p == 0), stop=(p == 3),
                                perf_mode=DR,
                            )
                e8s.append(e8)
                return zb

            def emit_recip(t, zb, recs):
                rec = rpool.tile([P, N], F32, tag="rec", name=f"rec{t}")
                for zt in range(2):
                    nc.vector.reciprocal(
                        rec[:, zt * 512:(zt + 1) * 512], zb[zt][:, :, 0:256]
                    )
                recs.append(rec)

            def emit_o(t, e8s, v8s, recs, last=False):
                """O matmuls + normalize + accumulate (+ store when last)."""
                e8, v8, rec = e8s[t], v8s[t], recs[t]
                for co in range(CC):
                    tmp = tpool.tile([P, N], F32, tag="tmp", name=f"tmp{t}{co}")
                    for ot in range(2):
                        op = zo.tile([P, 2, 512], F32, tag="zo",
                                     name=f"op{t}{co}{ot}")
                        for g in range(2):
                            nq = ot * 2 + g
                            for p in range(4):
                                nc.tensor.matmul(
                                    op[:, g, 0:256],
                                    v8[p // 2][:, 2 * (p % 2):2 * (p % 2) + 2,
                                               co * P:(co + 1) * P],
                                    e8[p][:, :, nq * 256:(nq + 1) * 256],
                                    start=(p == 0), stop=(p == 3),
                                    perf_mode=DR,
                                )
                        nc.vector.tensor_mul(
                            tmp[:, ot * 512:(ot + 1) * 512], op[:, :, 0:256],
                            rec[:, ot * 512:(ot + 1) * 512],
                        )
                    nc.gpsimd.tensor_add(acc[co], acc[co], tmp)
                    if last:
                        eng = nc.sync if co == 0 else nc.gpsimd
                        eng.dma_start(out=out_d[co], in_=acc[co])

            e8s, v8s, recs = [], [], []
            zb0 = emit_teacher(0, e8s, v8s, recs)
            # qg half drains were deferred to just after QG matmuls; DVE
            # order: qg drains come first (emitted below would be too late).
            emit_recip(0, zb0, recs)
            zb1 = emit_teacher(1, e8s, v8s, recs)
            emit_recip(1, zb1, recs)
            emit_o(0, e8s, v8s, recs)
            zb2 = emit_teacher(2, e8s, v8s, recs)
            emit_o(1, e8s, v8s, recs)
            emit_recip(2, zb2, recs)
            emit_o(2, e8s, v8s, recs, last=True)

    _split_multi_waits(nc)
    if not nc.is_finalized():
        nc.finalize()
    return nc


def _split_multi_waits(nc):
    """walrus can encode at most one sync-wait per instruction. Hoist every
    wait of a multi-wait instruction onto single-wait nops on the same
    engine, placed immediately before it in program order."""
    fixes = []
    for fn in nc.m.functions:
        for blk in fn.blocks:
            for inst in blk.instructions:
                si = getattr(inst, "sync_info", None)
                if (si is not None and si.on_wait and len(si.on_wait) > 1
                        and getattr(inst, "engine", None) is not None):
                    fixes.append((blk, inst))
    for blk, inst in fixes:
        si = inst.sync_info
        waits = list(si.on_wait)
        nops = []
        for w in waits:
            nop = nc.engines[inst.engine].nop(nofuse=True).ins
            nop.sync_info = mybir.SyncInfo(on_wait=[w], on_update=[])
            nops.append(nop)
        inst.sync_info = mybir.SyncInfo(on_wait=[], on_update=list(si.on_update))
        nop_names = {n.name for n in nops}
        for fn2 in nc.m.functions:
            for blk2 in fn2.blocks:
                blk2.instructions = [
                    i for i in blk2.instructions if i.name not in nop_names
                ]
        pos = next(i for i, x in enumerate(blk.instructions)
                   if x.name == inst.name)
        blk.instructions = (blk.instructions[:pos] + nops
                            + blk.instructions[pos:])


_NC = None


def _get_nc():
    global _NC
    if _NC is None:
        _NC = build_nc()
    return _NC


def make_in_maps(student_feat, t_feat0, t_feat1, t_feat2,
                 Wq, bq, Wk, bk, Wv, bv):
    xs = np.ascontiguousarray(
        student_feat.reshape(B, CC, P, N), dtype=np.float32)
    xt = np.ascontiguousarray(
        np.stack([t_feat0, t_feat1, t_feat2], axis=1).reshape(B, T, CC, P, N),
        dtype=np.float32)
    # S^T = Xt^T (G Xs + h); G = Wk^T Wq (x16 for fp8 range, folded back
    # via the exp scale), h = 16 Wk^T bq.  bk cancels in the key softmax.
    Wq64 = np.asarray(Wq, np.float64)
    Wk64 = np.asarray(Wk, np.float64)
    gT = ((Wq64.T @ Wk64) * 16.0).reshape(CC, P, C)
    hq = np.ascontiguousarray(
        ((Wk64.T @ np.asarray(bq, np.float64)) * 16.0).reshape(CC, P, 1),
        dtype=np.float32)
    wvT = Wv.T.reshape(CC, P, C)
    bvc = np.ascontiguousarray(bv.reshape(CC, P, 1), dtype=np.float32)
    # host-side fp8 quantization (bit-identical to a device tensor_copy)
    xs8 = np.ascontiguousarray(xs.astype(F8NP))
    xt8 = np.ascontiguousarray(xt.astype(F8NP))
    g8 = np.ascontiguousarray(np.asarray(gT, np.float32).astype(F8NP))
    wv8 = np.ascontiguousarray(np.asarray(wvT, np.float32).astype(F8NP))
    return [
        {"xs": xs[b], "xs8": xs8[b], "xt8": xt8[b], "g8": g8, "wv8": wv8,
         "hq": hq, "bv": bvc}
        for b in range(B)
    ]


def run(in_maps, trace=False):
    nc = _get_nc()
    return run_bass_kernel_spmd(nc, in_maps, core_ids=list(range(B)),
                                trace=trace)


def kernel(student_feat, t_feat0, t_feat1, t_feat2,
           Wq, bq, Wk, bk, Wv, bv):
    in_maps = make_in_maps(student_feat, t_feat0, t_feat1, t_feat2,
                           Wq, bq, Wk, bk, Wv, bv)
    res = run(in_maps, trace=False)
    out = np.stack([res.results[b]["out"].reshape(C, H, W) for b in range(B)])
    return out.astype(np.float32)


# revision 16
# speedup vs baseline: 1.1489x; 1.1489x over previous
"""CrossTeacherAttention Trainium2 kernel (fp8 DoubleRow + folded-QK).

Math per batch element b (x as [C=256, N=1024], N=H*W):
  S_t^T[m,n] = sum_c K_t[c,m] Q[c,n] with K_t = Wk Xt_t + bk, Q = Wq Xs + bq.
  Associativity folds Wk into Wq:  S_t^T = Xt_t^T @ QG,
    QG = G Xs + h,  G = Wk^T Wq (host-precomputed, x16 for fp8 range),
    h = 16 * Wk^T bq.  bk adds a per-column constant to the logits, which
  cancels exactly in the softmax over keys -> dropped.
  E_t = exp(S_t^T/256 - 4.5)  (offset keeps E in fp8e4 range; cancels in
  the O/Z ratio).  Zb_t = 3 * colsum(E_t), broadcast to 128 partitions by
  an all-3.0 stationary matmul (3 = the uniform 1/3 teacher weight:
  attn.mean(-1) of a softmax is exactly 1/N, so the cross-teacher softmax
  weights are uniform).  rec_t = 1/Zb_t.
  V_t^T from Xt_t^T @ Wv^T (bv folded into the acc init).
  out = Xs + bv + sum_t (V_t^T' E_t) * rec_t.

All matmuls are fp8e4 MatmulPerfMode.DoubleRow: one instruction
contracts 2x128=256 at 0.5 cycles/row.  fp8 operands (xs8/xt8/g8/wv8)
are quantized host-side (bit-identical to a device-side cast) so no
cast traffic or f32 teacher loads hit the device.  Engine split: Act
does only the 24 exps (the bottleneck), DVE does the PSUM drains (QG,
V) + reciprocals + normalize muls (gpsimd cannot touch PSUM on TRN2),
Pool (gpsimd) does acc init + accumulate adds + one output DMA queue.
Software pipeline: V[t] and S[t] emitted back to back, O[t] deferred
until after S[t+1] so the Act exp stream never waits on PE.

Sharding: data-parallel over batch, B=8 -> one batch element per core.
"""

import sys

sys.path.insert(0, "/opt/trn_rl_repo")

import numpy as np

import concourse.bass as bass
import concourse.tile as tile
from concourse import mybir
from concourse.bass_utils import run_bass_kernel_spmd

B, C, H, W = 8, 256, 32, 32
N = H * W  # 1024
T = 3
P = 128
CC = C // P  # 2 c-chunks
F32 = mybir.dt.float32
F8 = mybir.dt.float8e4
F8NP = mybir.dt.np(F8)
SCALE = 1.0 / 256.0  # 1/16 attention scale, 1/16 from the G x16 prescale
C0 = -4.5  # logit offset so exp() fits fp8e4 range
DR = mybir.MatmulPerfMode.DoubleRow
Exp = mybir.ActivationFunctionType.Exp


def build_nc():
    nc = bass.Bass()
    # all fp8/f32 operands partition-major so each loads in ONE DMA
    xs_d = nc.dram_tensor("xs", [P, CC, N], F32, kind="ExternalInput")
    xs8_d = nc.dram_tensor("xs8", [P, CC, N], F8, kind="ExternalInput")
    xt8_d = nc.dram_tensor("xt8", [T, P, CC, N], F8, kind="ExternalInput")
    g8_d = nc.dram_tensor("g8", [P, CC, C], F8, kind="ExternalInput")
    wv8_d = nc.dram_tensor("wv8", [P, CC, C], F8, kind="ExternalInput")
    hv_d = nc.dram_tensor("hv", [P, CC, 2], F32, kind="ExternalInput")
    out_d = nc.dram_tensor("out", [CC, P, N], F32, kind="ExternalOutput")

    with tile.TileContext(nc) as tc:
        with (
            tc.tile_pool(name="consts", bufs=1) as consts,
            tc.tile_pool(name="epool", bufs=8) as epool,
            tc.tile_pool(name="vpool", bufs=4) as vpool,
            tc.tile_pool(name="rpool", bufs=2) as rpool,
            tc.tile_pool(name="tpool", bufs=2) as tpool,
            tc.tile_pool(name="mm", bufs=2, space="PSUM") as mm,
            tc.tile_pool(name="zo", bufs=2, space="PSUM") as zo,
        ):
            # ---- loads: one DMA per tensor, criticality-ordered on SP;
            #      big/late loads go on the Pool DGE queue ----
            g8 = consts.tile([P, CC, C], F8, tag="g8", name="g8")
            nc.sync.dma_start(out=g8, in_=g8_d[:, :, :])
            xs8 = consts.tile([P, CC, N], F8, tag="xs8", name="xs8")
            nc.sync.dma_start(out=xs8, in_=xs8_d[:, :, :])
            xt8 = [consts.tile([P, CC, N], F8, tag=f"xt8_{t}", name=f"xt8_{t}")
                   for t in range(T)]
            nc.sync.dma_start(out=xt8[0], in_=xt8_d[0])
            wv8 = consts.tile([P, CC, C], F8, tag="wv8", name="wv8")
            nc.sync.dma_start(out=wv8, in_=wv8_d[:, :, :])
            hv_sb = consts.tile([P, CC, 2], F32, tag="hv", name="hv")
            nc.sync.dma_start(out=hv_sb, in_=hv_d[:, :, :])
            xs_sb = consts.tile([P, CC, N], F32, tag="xs", name="xs")
            nc.gpsimd.dma_start(out=xs_sb, in_=xs_d[:, :, :])
            for t in range(1, T):
                nc.gpsimd.dma_start(out=xt8[t], in_=xt8_d[t])

            ones8 = consts.tile([P, CC, P], F8, tag="ones8", name="ones8")
            nc.gpsimd.memset(ones8, 3.0)
            c0_sb = consts.tile([P, 1], F32, tag="c0", name="c0")
            nc.gpsimd.memset(c0_sb, C0)
            # warm the Exp activation table while the pipeline fills
            warm = consts.tile([P, 1], F32, tag="warm", name="warm")
            nc.scalar.activation(warm, c0_sb, func=Exp, scale=1.0)

            # ---- QG projection (fp8 DoubleRow), drained per (co, half) ----
            qg8 = consts.tile([P, CC, N], F8, tag="qg8", name="qg8")
            qps = []
            for co in range(CC):
                qp = mm.tile([P, N], F32, tag="mm", name=f"qp{co}")
                for nq in range(4):
                    nc.tensor.matmul(
                        qp[:, nq * 256:(nq + 1) * 256],
                        g8[:, :, co * P:(co + 1) * P],
                        xs8[:, :, nq * 256:(nq + 1) * 256],
                        start=True, stop=True, perf_mode=DR,
                    )
                qps.append(qp)
            # half 0 of both co first (gates S mi=0's first matmuls)
            for hh in range(2):
                sl = slice(hh * 512, (hh + 1) * 512)
                for co in range(CC):
                    nc.vector.tensor_scalar_add(
                        qg8[:, co, sl], qps[co][:, sl], hv_sb[:, co, 0:1])

            # ---- acc init: acc = xs + bv (Pool) ----
            acc = []
            for co in range(CC):
                a_ = consts.tile([P, N], F32, tag=f"acc{co}", name=f"acc{co}")
                nc.gpsimd.tensor_scalar_add(a_, xs_sb[:, co, :],
                                            hv_sb[:, co, 1:2])
                acc.append(a_)

            def emit_teacher(t, e8s, v8s, recs):
                """V proj + S/exp/Zb + recip for teacher t."""
                # V^T projection -> v8 half tiles [P(m), (mi in half), c]
                v8 = [vpool.tile([P, 4, C], F8, tag="v", name=f"v{t}{h}")
                      for h in range(2)]
                for h in range(2):
                    vp = zo.tile([P, 2, 512], F32, tag="zo", name=f"vp{t}{h}")
                    for q in range(4):
                        mi = h * 4 + q
                        nc.tensor.matmul(
                            vp[:, q // 2, (q % 2) * 256:(q % 2) * 256 + 256],
                            xt8[t][:, :, mi * P:(mi + 1) * P],
                            wv8[:, :, :],
                            start=True, stop=True, perf_mode=DR,
                        )
                    nc.vector.tensor_copy(v8[h], vp)
                v8s.append(v8)

                # S + exp, Zb matmuls interleaved per pair
                zb = [zo.tile([P, 2, 512], F32, tag="zo", name=f"zb{t}{zt}")
                      for zt in range(2)]
                e8 = [epool.tile([P, 2, N], F8, tag="e", name=f"e{t}{p}")
                      for p in range(4)]
                for mi in range(8):
                    p, j = mi // 2, mi % 2
                    sp = mm.tile([P, N], F32, tag="mm", name=f"sp{t}{mi}")
                    for nq in range(4):
                        nc.tensor.matmul(
                            sp[:, nq * 256:(nq + 1) * 256],
                            xt8[t][:, :, mi * P:(mi + 1) * P],
                            qg8[:, :, nq * 256:(nq + 1) * 256],
                            start=True, stop=True, perf_mode=DR,
                        )
                    nc.scalar.activation(e8[p][:, j, :], sp, func=Exp,
                                         scale=SCALE, bias=c0_sb)
                    if j == 1:
                        for nq in range(4):
                            nc.tensor.matmul(
                                zb[nq // 2][:, nq % 2, 0:256],
                                ones8,
                                e8[p][:, :, nq * 256:(nq + 1) * 256],
                                start=(p == 0), stop=(p == 3),
                                perf_mode=DR,
                            )
                e8s.append(e8)
                return zb

            def emit_recip(t, zb, recs):
                rec = rpool.tile([P, N], F32, tag="rec", name=f"rec{t}")
                for zt in range(2):
                    nc.vector.reciprocal(
                        rec[:, zt * 512:(zt + 1) * 512], zb[zt][:, :, 0:256]
                    )
                recs.append(rec)

            def emit_o(t, e8s, v8s, recs, last=False):
                """O matmuls + normalize + accumulate (+ store when last)."""
                e8, v8, rec = e8s[t], v8s[t], recs[t]
                for co in range(CC):
                    tmp = tpool.tile([P, N], F32, tag="tmp", name=f"tmp{t}{co}")
                    for ot in range(2):
                        op = zo.tile([P, 2, 512], F32, tag="zo",
                                     name=f"op{t}{co}{ot}")
                        for g in range(2):
                            nq = ot * 2 + g
                            for p in range(4):
                                nc.tensor.matmul(
                                    op[:, g, 0:256],
                                    v8[p // 2][:, 2 * (p % 2):2 * (p % 2) + 2,
                                               co * P:(co + 1) * P],
                                    e8[p][:, :, nq * 256:(nq + 1) * 256],
                                    start=(p == 0), stop=(p == 3),
                                    perf_mode=DR,
                                )
                        nc.vector.tensor_mul(
                            tmp[:, ot * 512:(ot + 1) * 512], op[:, :, 0:256],
                            rec[:, ot * 512:(ot + 1) * 512],
                        )
                    nc.gpsimd.tensor_add(acc[co], acc[co], tmp)
                    if last:
                        eng = nc.sync if co == 0 else nc.gpsimd
                        eng.dma_start(out=out_d[co], in_=acc[co])

            e8s, v8s, recs = [], [], []
            zb0 = emit_teacher(0, e8s, v8s, recs)
            # qg half drains were deferred to just after QG matmuls; DVE
            # order: qg drains come first (emitted below would be too late).
            emit_recip(0, zb0, recs)
            zb1 = emit_teacher(1, e8s, v8s, recs)
            emit_recip(1, zb1, recs)
            emit_o(0, e8s, v8s, recs)
            zb2 = emit_teacher(2, e8s, v8s, recs)
            emit_o(1, e8s, v8s, recs)
            emit_recip(2, zb2, recs)
            emit_o(2, e8s, v8s, recs, last=True)

    _split_multi_waits(nc)
    if not nc.is_finalized():
        nc.finalize()
    return nc


def _split_multi_waits(nc):
    """walrus can encode at most one sync-wait per instruction. Hoist every
    wait of a multi-wait instruction onto single-wait nops on the same
    engine, placed immediately before it in program order."""
    fixes = []
    for fn in nc.m.functions:
        for blk in fn.blocks:
            for inst in blk.instructions:
                si = getattr(inst, "sync_info", None)
                if (si is not None and si.on_wait and len(si.on_wait) > 1
                        and getattr(inst, "engine", None) is not None):
                    fixes.append((blk, inst))
    for blk, inst in fixes:
        si = inst.sync_info
        waits = list(si.on_wait)
        nops = []
        for w in waits:
            nop = nc.engines[inst.engine].nop(nofuse=True).ins
            nop.sync_info = mybir.SyncInfo(on_wait=[w], on_update=[])
            nops.append(nop)
        inst.sync_info = mybir.SyncInfo(on_wait=[], on_update=list(si.on_update))
        nop_names = {n.name for n in nops}
        for fn2 in nc.m.functions:
            for blk2 in fn2.blocks:
                blk2.instructions = [
                    i for i in blk2.instructions if i.name not in nop_names
                ]
        pos = next(i for i, x in enumerate(blk.instructions)
                   if x.name == inst.name)
        blk.instructions = (blk.instructions[:pos] + nops
                            + blk.instructions[pos:])


_NC = None


def _get_nc():
    global _NC
    if _NC is None:
        _NC = build_nc()
    return _NC


def make_in_maps(student_feat, t_feat0, t_feat1, t_feat2,
                 Wq, bq, Wk, bk, Wv, bv):
    # partition-major: [.., CC, P, ..] -> [.., P, CC, ..] so each tensor
    # loads in a single DMA
    xs = np.ascontiguousarray(
        student_feat.reshape(B, CC, P, N).transpose(0, 2, 1, 3),
        dtype=np.float32)
    xt = np.stack([t_feat0, t_feat1, t_feat2], axis=1).reshape(
        B, T, CC, P, N).transpose(0, 1, 3, 2, 4)
    # S^T = Xt^T (G Xs + h); G = Wk^T Wq (x16 for fp8 range, folded back
    # via the exp scale), h = 16 Wk^T bq.  bk cancels in the key softmax.
    Wq64 = np.asarray(Wq, np.float64)
    Wk64 = np.asarray(Wk, np.float64)
    gT = ((Wq64.T @ Wk64) * 16.0).reshape(CC, P, C).transpose(1, 0, 2)
    hq = ((Wk64.T @ np.asarray(bq, np.float64)) * 16.0).reshape(CC, P, 1)
    wvT = Wv.T.reshape(CC, P, C).transpose(1, 0, 2)
    hv = np.ascontiguousarray(
        np.concatenate([hq, bv.reshape(CC, P, 1)], axis=2).transpose(1, 0, 2),
        dtype=np.float32)
    # host-side fp8 quantization (bit-identical to a device tensor_copy)
    xs8 = np.ascontiguousarray(np.asarray(xs, np.float32).astype(F8NP))
    xt8 = np.ascontiguousarray(np.asarray(xt, np.float32).astype(F8NP))
    g8 = np.ascontiguousarray(np.asarray(gT, np.float32).astype(F8NP))
    wv8 = np.ascontiguousarray(np.asarray(wvT, np.float32).astype(F8NP))
    return [
        {"xs": xs[b], "xs8": xs8[b], "xt8": xt8[b], "g8": g8, "wv8": wv8,
         "hv": hv}
        for b in range(B)
    ]


def run(in_maps, trace=False):
    nc = _get_nc()
    return run_bass_kernel_spmd(nc, in_maps, core_ids=list(range(B)),
                                trace=trace)


def kernel(student_feat, t_feat0, t_feat1, t_feat2,
           Wq, bq, Wk, bk, Wv, bv):
    in_maps = make_in_maps(student_feat, t_feat0, t_feat1, t_feat2,
                           Wq, bq, Wk, bk, Wv, bv)
    res = run(in_maps, trace=False)
    out = np.stack([res.results[b]["out"].reshape(C, H, W) for b in range(B)])
    return out.astype(np.float32)


# revision 17
# speedup vs baseline: 1.4411x; 1.2543x over previous
"""CrossTeacherAttention Trainium2 kernel (fp8 DoubleRow + folded-QK).

Math per batch element b (x as [C=256, N=1024], N=H*W):
  S_t^T[m,n] = sum_c K_t[c,m] Q[c,n] with K_t = Wk Xt_t + bk, Q = Wq Xs + bq.
  Associativity folds Wk into Wq:  S_t^T = Xt_t^T @ QG,
    QG = G Xs + h,  G = Wk^T Wq (host-precomputed, x16 for fp8 range),
    h = 16 * Wk^T bq.  bk adds a per-column constant to the logits, which
  cancels exactly in the softmax over keys -> dropped.
  E_t = exp(S_t^T/256 - 4.5)  (offset keeps E in fp8e4 range; cancels in
  the O/Z ratio).  Zb_t = 3 * colsum(E_t), broadcast to 128 partitions by
  an all-3.0 stationary matmul (3 = the uniform 1/3 teacher weight:
  attn.mean(-1) of a softmax is exactly 1/N, so the cross-teacher softmax
  weights are uniform).  rec_t = 1/Zb_t.
  V_t^T from Xt_t^T @ Wv^T (bv folded into the acc init).
  out = Xs + bv + sum_t (V_t^T' E_t) * rec_t.

All matmuls are fp8e4 MatmulPerfMode.DoubleRow: one instruction
contracts 2x128=256 at 0.5 cycles/row.  fp8 operands (xs8/xt8/g8/wv8)
are quantized host-side (bit-identical to a device-side cast) so no
cast traffic or f32 teacher loads hit the device.  Engine split: Act
does only the 24 exps (the bottleneck), DVE does the PSUM drains (QG,
V) + reciprocals + normalize muls (gpsimd cannot touch PSUM on TRN2),
Pool (gpsimd) does acc init + accumulate adds + one output DMA queue.
Software pipeline: V[t] and S[t] emitted back to back, O[t] deferred
until after S[t+1] so the Act exp stream never waits on PE.

Sharding: data-parallel over batch, B=8 -> one batch element per core.
"""

import sys

sys.path.insert(0, "/opt/trn_rl_repo")

import numpy as np

import concourse.bass as bass
import concourse.tile as tile
from concourse import mybir
from concourse.bass_utils import run_bass_kernel_spmd

B, C, H, W = 8, 256, 32, 32
N = H * W  # 1024
T = 3
P = 128
CC = C // P  # 2 c-chunks
F32 = mybir.dt.float32
F8 = mybir.dt.float8e4
F8NP = mybir.dt.np(F8)
SCALE = 1.0 / 256.0  # 1/16 attention scale, 1/16 from the G x16 prescale
C0 = -4.5  # logit offset so exp() fits fp8e4 range
DR = mybir.MatmulPerfMode.DoubleRow
Exp = mybir.ActivationFunctionType.Exp


def build_nc():
    nc = bass.Bass()
    # all fp8/f32 operands partition-major so each loads in ONE DMA
    xsbv_d = nc.dram_tensor("xsbv", [P, CC, N], F32, kind="ExternalInput")
    xs8_d = nc.dram_tensor("xs8", [P, CC, N], F8, kind="ExternalInput")
    xt8_d = nc.dram_tensor("xt8", [T, P, CC, N], F8, kind="ExternalInput")
    g8_d = nc.dram_tensor("g8", [P, CC, C], F8, kind="ExternalInput")
    wv8_d = nc.dram_tensor("wv8", [P, CC, C], F8, kind="ExternalInput")
    hq_d = nc.dram_tensor("hq", [P, CC, 1], F32, kind="ExternalInput")
    out_d = nc.dram_tensor("out", [CC, P, N], F32, kind="ExternalOutput")

    with tile.TileContext(nc) as tc:
        with (
            tc.tile_pool(name="consts", bufs=1) as consts,
            tc.tile_pool(name="epool", bufs=12) as epool,
            tc.tile_pool(name="vpool", bufs=4) as vpool,
            tc.tile_pool(name="rpool", bufs=2) as rpool,
            tc.tile_pool(name="tpool", bufs=2) as tpool,
            tc.tile_pool(name="mm", bufs=2, space="PSUM") as mm,
            tc.tile_pool(name="zo", bufs=2, space="PSUM") as zo,
        ):
            # ---- memsets first so the Pool queue starts instantly ----
            ones8 = consts.tile([P, CC, P], F8, tag="ones8", name="ones8")
            nc.gpsimd.memset(ones8, 3.0)
            c0_sb = consts.tile([P, 1], F32, tag="c0", name="c0")
            nc.gpsimd.memset(c0_sb, C0)
            # warm the Exp activation table while the pipeline fills
            warm = consts.tile([P, 1], F32, tag="warm", name="warm")
            nc.scalar.activation(warm, c0_sb, func=Exp, scale=1.0)

            # ---- loads: one DMA per tensor, criticality-ordered.
            #      SP queue: xs8, g8, xt8[0] (gate the first S matmuls).
            #      Pool queue: hq, wv8, acc(=xs+bv), xt8[1], xt8[2]. ----
            xs8 = consts.tile([P, CC, N], F8, tag="xs8", name="xs8")
            nc.sync.dma_start(out=xs8, in_=xs8_d[:, :, :])
            g8 = consts.tile([P, CC, C], F8, tag="g8", name="g8")
            nc.sync.dma_start(out=g8, in_=g8_d[:, :, :])
            xt8 = [consts.tile([P, CC, N], F8, tag=f"xt8_{t}", name=f"xt8_{t}")
                   for t in range(T)]
            nc.sync.dma_start(out=xt8[0], in_=xt8_d[0])
            hq_sb = consts.tile([P, CC, 1], F32, tag="hq", name="hq")
            nc.gpsimd.dma_start(out=hq_sb, in_=hq_d[:, :, :])
            wv8 = consts.tile([P, CC, C], F8, tag="wv8", name="wv8")
            nc.gpsimd.dma_start(out=wv8, in_=wv8_d[:, :, :])
            acc = consts.tile([P, CC, N], F32, tag="acc", name="acc")
            nc.gpsimd.dma_start(out=acc, in_=xsbv_d[:, :, :])
            for t in range(1, T):
                nc.gpsimd.dma_start(out=xt8[t], in_=xt8_d[t])

            # ---- QG projection (fp8 DoubleRow); qp tiles live in the zo
            #      pool so the mm pool is sp-only; drains split Act/DVE ----
            qg8 = consts.tile([P, CC, N], F8, tag="qg8", name="qg8")
            qps = []
            for co in range(CC):
                qp = zo.tile([P, 2, 512], F32, tag="zo", name=f"qp{co}")
                for nq in range(4):
                    nc.tensor.matmul(
                        qp[:, nq // 2, (nq % 2) * 256:(nq % 2) * 256 + 256],
                        g8[:, :, co * P:(co + 1) * P],
                        xs8[:, :, nq * 256:(nq + 1) * 256],
                        start=True, stop=True, perf_mode=DR,
                    )
                qps.append(qp)
            nc.scalar.add(qg8[:, 0, :], qps[0][:, :, :], hq_sb[:, 0, :])
            nc.vector.tensor_scalar_add(qg8[:, 1, :], qps[1][:, :, :],
                                        hq_sb[:, 1, :])

            def emit_v(t, v8s):
                """V^T projection -> v8 half tiles [P(m), (mi in half), c]"""
                v8 = [vpool.tile([P, 4, C], F8, tag="v", name=f"v{t}{h}")
                      for h in range(2)]
                for h in range(2):
                    vp = zo.tile([P, 2, 512], F32, tag="zo", name=f"vp{t}{h}")
                    for q in range(4):
                        mi = h * 4 + q
                        nc.tensor.matmul(
                            vp[:, q // 2, (q % 2) * 256:(q % 2) * 256 + 256],
                            xt8[t][:, :, mi * P:(mi + 1) * P],
                            wv8[:, :, :],
                            start=True, stop=True, perf_mode=DR,
                        )
                    nc.vector.tensor_copy(v8[h], vp)
                v8s.append(v8)

            def emit_s_mi(t, mi, e8):
                """S matmuls + exp for one m-chunk of teacher t."""
                p, j = mi // 2, mi % 2
                sp = mm.tile([P, N], F32, tag="mm", name=f"sp{t}{mi}")
                for nq in range(4):
                    nc.tensor.matmul(
                        sp[:, nq * 256:(nq + 1) * 256],
                        xt8[t][:, :, mi * P:(mi + 1) * P],
                        qg8[:, :, nq * 256:(nq + 1) * 256],
                        start=True, stop=True, perf_mode=DR,
                    )
                nc.scalar.activation(e8[p][:, j, :], sp, func=Exp,
                                     scale=SCALE, bias=c0_sb)

            def emit_zb(t, e8, recs):
                """Zb burst (3*colsum(E) broadcast) + reciprocal."""
                rec = rpool.tile([P, N], F32, tag="rec", name=f"rec{t}")
                for zt in range(2):
                    zb = zo.tile([P, 2, 512], F32, tag="zo",
                                 name=f"zb{t}{zt}")
                    for g in range(2):
                        nq = zt * 2 + g
                        for p in range(4):
                            nc.tensor.matmul(
                                zb[:, g, 0:256],
                                ones8,
                                e8[p][:, :, nq * 256:(nq + 1) * 256],
                                start=(p == 0), stop=(p == 3),
                                perf_mode=DR,
                            )
                    nc.vector.reciprocal(
                        rec[:, zt * 512:(zt + 1) * 512], zb[:, :, 0:256])
                recs.append(rec)

            def emit_o_chunk(t, co, ot, e8, v8, rec, tmp):
                """One (co, ot) O chunk: 8 matmuls + normalize mul."""
                op = zo.tile([P, 2, 512], F32, tag="zo", name=f"op{t}{co}{ot}")
                for g in range(2):
                    nq = ot * 2 + g
                    for p in range(4):
                        nc.tensor.matmul(
                            op[:, g, 0:256],
                            v8[p // 2][:, 2 * (p % 2):2 * (p % 2) + 2,
                                       co * P:(co + 1) * P],
                            e8[p][:, :, nq * 256:(nq + 1) * 256],
                            start=(p == 0), stop=(p == 3),
                            perf_mode=DR,
                        )
                nc.vector.tensor_mul(
                    tmp[:, ot * 512:(ot + 1) * 512], op[:, :, 0:256],
                    rec[:, ot * 512:(ot + 1) * 512],
                )

            e8s, v8s, recs = [], [], []
            for t in range(T):
                e8s.append([epool.tile([P, 2, N], F8, tag="e",
                                       name=f"e{t}{p}") for p in range(4)])

            # teacher 0: V + S (exp stream starts here)
            emit_v(0, v8s)
            for mi in range(8):
                emit_s_mi(0, mi, e8s[0])

            # teachers 1, 2: V + S with prior teacher's Zb/O interleaved
            # into the S mi-loop so PE work fills the Act-bound stalls
            for t in (1, 2):
                pt = t - 1
                emit_v(t, v8s)
                emit_s_mi(t, 0, e8s[t])
                emit_zb(pt, e8s[pt], recs)
                tmp = tpool.tile([P, N], F32, tag="tmp", name=f"tmp{pt}")
                emit_s_mi(t, 1, e8s[t])
                emit_o_chunk(pt, 0, 0, e8s[pt], v8s[pt], recs[pt], tmp)
                emit_s_mi(t, 2, e8s[t])
                emit_o_chunk(pt, 0, 1, e8s[pt], v8s[pt], recs[pt], tmp)
                emit_s_mi(t, 3, e8s[t])
                nc.gpsimd.tensor_add(acc[:, 0, :], acc[:, 0, :], tmp)
                emit_o_chunk(pt, 1, 0, e8s[pt], v8s[pt], recs[pt], tmp)
                emit_s_mi(t, 4, e8s[t])
                emit_o_chunk(pt, 1, 1, e8s[pt], v8s[pt], recs[pt], tmp)
                emit_s_mi(t, 5, e8s[t])
                nc.gpsimd.tensor_add(acc[:, 1, :], acc[:, 1, :], tmp)
                for mi in range(6, 8):
                    emit_s_mi(t, mi, e8s[t])

            # teacher 2 tail: Zb burst, O chunks, per-chunk add + store
            emit_zb(2, e8s[2], recs)
            tmp = tpool.tile([P, N], F32, tag="tmp", name="tmp2")
            for co in range(CC):
                for ot in range(2):
                    emit_o_chunk(2, co, ot, e8s[2], v8s[2], recs[2], tmp)
                    sl = slice(ot * 512, (ot + 1) * 512)
                    nc.gpsimd.tensor_add(acc[:, co, sl], acc[:, co, sl],
                                         tmp[:, sl])
                    eng = nc.sync if co == 0 else nc.gpsimd
                    eng.dma_start(out=out_d[co][:, sl], in_=acc[:, co, sl])

    _split_multi_waits(nc)
    if not nc.is_finalized():
        nc.finalize()
    return nc


def _split_multi_waits(nc):
    """walrus can encode at most one sync-wait per instruction. Hoist every
    wait of a multi-wait instruction onto single-wait nops on the same
    engine, placed immediately before it in program order."""
    fixes = []
    for fn in nc.m.functions:
        for blk in fn.blocks:
            for inst in blk.instructions:
                si = getattr(inst, "sync_info", None)
                if (si is not None and si.on_wait and len(si.on_wait) > 1
                        and getattr(inst, "engine", None) is not None):
                    fixes.append((blk, inst))
    for blk, inst in fixes:
        si = inst.sync_info
        waits = list(si.on_wait)
        nops = []
        for w in waits:
            nop = nc.engines[inst.engine].nop(nofuse=True).ins
            nop.sync_info = mybir.SyncInfo(on_wait=[w], on_update=[])
            nops.append(nop)
        inst.sync_info = mybir.SyncInfo(on_wait=[], on_update=list(si.on_update))
        nop_names = {n.name for n in nops}
        for fn2 in nc.m.functions:
            for blk2 in fn2.blocks:
                blk2.instructions = [
                    i for i in blk2.instructions if i.name not in nop_names
                ]
        pos = next(i for i, x in enumerate(blk.instructions)
                   if x.name == inst.name)
        blk.instructions = (blk.instructions[:pos] + nops
                            + blk.instructions[pos:])


_NC = None


def _get_nc():
    global _NC
    if _NC is None:
        _NC = build_nc()
    return _NC


def make_in_maps(student_feat, t_feat0, t_feat1, t_feat2,
                 Wq, bq, Wk, bk, Wv, bv):
    # partition-major: [.., CC, P, ..] -> [.., P, CC, ..] so each tensor
    # loads in a single DMA
    xs = np.ascontiguousarray(
        student_feat.reshape(B, CC, P, N).transpose(0, 2, 1, 3),
        dtype=np.float32)
    xt = np.stack([t_feat0, t_feat1, t_feat2], axis=1).reshape(
        B, T, CC, P, N).transpose(0, 1, 3, 2, 4)
    # S^T = Xt^T (G Xs + h); G = Wk^T Wq (x16 for fp8 range, folded back
    # via the exp scale), h = 16 Wk^T bq.  bk cancels in the key softmax.
    Wq64 = np.asarray(Wq, np.float64)
    Wk64 = np.asarray(Wk, np.float64)
    gT = ((Wq64.T @ Wk64) * 16.0).reshape(CC, P, C).transpose(1, 0, 2)
    hq = ((Wk64.T @ np.asarray(bq, np.float64)) * 16.0).reshape(CC, P, 1)
    wvT = Wv.T.reshape(CC, P, C).transpose(1, 0, 2)
    hqc = np.ascontiguousarray(hq.transpose(1, 0, 2), dtype=np.float32)
    xsbv = np.ascontiguousarray(
        xs + np.asarray(bv, np.float32).reshape(CC, P, 1)
        .transpose(1, 0, 2)[None, :, :, :] * np.ones((B, 1, 1, 1), np.float32),
        dtype=np.float32)
    # host-side fp8 quantization (bit-identical to a device tensor_copy)
    xs8 = np.ascontiguousarray(np.asarray(xs, np.float32).astype(F8NP))
    xt8 = np.ascontiguousarray(np.asarray(xt, np.float32).astype(F8NP))
    g8 = np.ascontiguousarray(np.asarray(gT, np.float32).astype(F8NP))
    wv8 = np.ascontiguousarray(np.asarray(wvT, np.float32).astype(F8NP))
    return [
        {"xsbv": xsbv[b], "xs8": xs8[b], "xt8": xt8[b], "g8": g8, "wv8": wv8,
         "hq": hqc}
        for b in range(B)
    ]


def run(in_maps, trace=False):
    nc = _get_nc()
    return run_bass_kernel_spmd(nc, in_maps, core_ids=list(range(B)),
                                trace=trace)


def kernel(student_feat, t_feat0, t_feat1, t_feat2,
           Wq, bq, Wk, bk, Wv, bv):
    in_maps = make_in_maps(student_feat, t_feat0, t_feat1, t_feat2,
                           Wq, bq, Wk, bk, Wv, bv)
    res = run(in_maps, trace=False)
    out = np.stack([res.results[b]["out"].reshape(C, H, W) for b in range(B)])
    return out.astype(np.float32)


# revision 19
# speedup vs baseline: 1.4621x; 1.0146x over previous
"""CrossTeacherAttention Trainium2 kernel (fp8 DoubleRow + folded-QK).

Math per batch element b (x as [C=256, N=1024], N=H*W):
  S_t^T[m,n] = sum_c K_t[c,m] Q[c,n] with K_t = Wk Xt_t + bk, Q = Wq Xs + bq.
  Associativity folds Wk into Wq:  S_t^T = Xt_t^T @ QG,
    QG = G Xs + h,  G = Wk^T Wq (host-precomputed, x16 for fp8 range),
    h = 16 * Wk^T bq.  bk adds a per-column constant to the logits, which
  cancels exactly in the softmax over keys -> dropped.
  E_t = exp(S_t^T/256 - 4.5)  (offset keeps E in fp8e4 range; cancels in
  the O/Z ratio).  Zb_t = 3 * colsum(E_t), broadcast to 128 partitions by
  an all-3.0 stationary matmul (3 = the uniform 1/3 teacher weight:
  attn.mean(-1) of a softmax is exactly 1/N, so the cross-teacher softmax
  weights are uniform).  rec_t = 1/Zb_t.
  V_t^T from Xt_t^T @ Wv^T (bv folded into the acc init).
  out = Xs + bv + sum_t (V_t^T' E_t) * rec_t.

All matmuls are fp8e4 MatmulPerfMode.DoubleRow: one instruction
contracts 2x128=256 at 0.5 cycles/row.  fp8 operands (xs8/xt8/g8/wv8)
are quantized host-side (bit-identical to a device-side cast) so no
cast traffic or f32 teacher loads hit the device.  Engine split: Act
does only the 24 exps (the bottleneck), DVE does the PSUM drains (QG,
V) + reciprocals + normalize muls (gpsimd cannot touch PSUM on TRN2),
Pool (gpsimd) does acc init + accumulate adds + one output DMA queue.
Software pipeline: V[t] and S[t] emitted back to back, O[t] deferred
until after S[t+1] so the Act exp stream never waits on PE.

Sharding: data-parallel over batch, B=8 -> one batch element per core.
"""

import sys

sys.path.insert(0, "/opt/trn_rl_repo")

import numpy as np

import concourse.bass as bass
import concourse.tile as tile
from concourse import mybir
from concourse.bass_utils import run_bass_kernel_spmd

B, C, H, W = 8, 256, 32, 32
N = H * W  # 1024
T = 3
P = 128
CC = C // P  # 2 c-chunks
F32 = mybir.dt.float32
F8 = mybir.dt.float8e4
F8NP = mybir.dt.np(F8)
SCALE = 1.0 / 256.0  # 1/16 attention scale, 1/16 from the G x16 prescale
C0 = -4.5  # logit offset so exp() fits fp8e4 range
DR = mybir.MatmulPerfMode.DoubleRow
Exp = mybir.ActivationFunctionType.Exp


def build_nc():
    nc = bass.Bass()
    # all fp8/f32 operands partition-major so each loads in ONE DMA
    xsbv_d = nc.dram_tensor("xsbv", [P, CC, N], F32, kind="ExternalInput")
    xs8_d = nc.dram_tensor("xs8", [P, CC, N], F8, kind="ExternalInput")
    xt8_d = nc.dram_tensor("xt8", [T, P, CC, N], F8, kind="ExternalInput")
    g8_d = nc.dram_tensor("g8", [P, CC, C], F8, kind="ExternalInput")
    wv8_d = nc.dram_tensor("wv8", [P, CC, C], F8, kind="ExternalInput")
    hq_d = nc.dram_tensor("hq", [P, CC, 1], F32, kind="ExternalInput")
    out_d = nc.dram_tensor("out", [CC, P, N], F32, kind="ExternalOutput")

    with tile.TileContext(nc) as tc:
        with (
            tc.tile_pool(name="consts", bufs=1) as consts,
            tc.tile_pool(name="epool", bufs=12) as epool,
            tc.tile_pool(name="vpool", bufs=4) as vpool,
            tc.tile_pool(name="rpool", bufs=2) as rpool,
            tc.tile_pool(name="tpool", bufs=2) as tpool,
            tc.tile_pool(name="mm", bufs=2, space="PSUM") as mm,
            tc.tile_pool(name="zo", bufs=2, space="PSUM") as zo,
        ):
            # ---- memsets first so the Pool queue starts instantly ----
            ones8 = consts.tile([P, CC, P], F8, tag="ones8", name="ones8")
            nc.gpsimd.memset(ones8, 3.0)
            c0_sb = consts.tile([P, 1], F32, tag="c0", name="c0")
            nc.gpsimd.memset(c0_sb, C0)
            # warm the Exp activation table while the pipeline fills
            warm = consts.tile([P, 1], F32, tag="warm", name="warm")
            nc.scalar.activation(warm, c0_sb, func=Exp, scale=1.0)

            # ---- loads: one DMA per tensor, criticality-ordered.
            #      SP queue: xs8, g8, xt8[0] (gate the first S matmuls).
            #      Pool queue: hq, wv8, acc(=xs+bv), xt8[1], xt8[2]. ----
            xs8 = consts.tile([P, CC, N], F8, tag="xs8", name="xs8")
            nc.sync.dma_start(out=xs8, in_=xs8_d[:, :, :])
            g8 = consts.tile([P, CC, C], F8, tag="g8", name="g8")
            nc.sync.dma_start(out=g8, in_=g8_d[:, :, :])
            xt8 = [consts.tile([P, CC, N], F8, tag=f"xt8_{t}", name=f"xt8_{t}")
                   for t in range(T)]
            nc.sync.dma_start(out=xt8[0], in_=xt8_d[0])
            hq_sb = consts.tile([P, CC, 1], F32, tag="hq", name="hq")
            nc.gpsimd.dma_start(out=hq_sb, in_=hq_d[:, :, :])
            wv8 = consts.tile([P, CC, C], F8, tag="wv8", name="wv8")
            nc.gpsimd.dma_start(out=wv8, in_=wv8_d[:, :, :])
            acc = consts.tile([P, CC, N], F32, tag="acc", name="acc")
            nc.gpsimd.dma_start(out=acc, in_=xsbv_d[:, :, :])
            for t in range(1, T):
                nc.gpsimd.dma_start(out=xt8[t], in_=xt8_d[t])

            # ---- QG projection (fp8 DoubleRow); qp tiles live in the zo
            #      pool so the mm pool is sp-only; drains split Act/DVE ----
            qg8 = consts.tile([P, CC, N], F8, tag="qg8", name="qg8")
            qps = []
            for co in range(CC):
                qp = zo.tile([P, 2, 512], F32, tag="zo", name=f"qp{co}")
                for nq in range(4):
                    nc.tensor.matmul(
                        qp[:, nq // 2, (nq % 2) * 256:(nq % 2) * 256 + 256],
                        g8[:, :, co * P:(co + 1) * P],
                        xs8[:, :, nq * 256:(nq + 1) * 256],
                        start=True, stop=True, perf_mode=DR,
                    )
                qps.append(qp)
            nc.scalar.add(qg8[:, 0, :], qps[0][:, :, :], hq_sb[:, 0, :])
            nc.vector.tensor_scalar_add(qg8[:, 1, :], qps[1][:, :, :],
                                        hq_sb[:, 1, :])

            def emit_v(t, v8s):
                """V^T projection -> v8 half tiles [P(m), (mi in half), c]"""
                v8 = [vpool.tile([P, 4, C], F8, tag="v", name=f"v{t}{h}")
                      for h in range(2)]
                for h in range(2):
                    vp = zo.tile([P, 2, 512], F32, tag="zo", name=f"vp{t}{h}")
                    for q in range(4):
                        mi = h * 4 + q
                        nc.tensor.matmul(
                            vp[:, q // 2, (q % 2) * 256:(q % 2) * 256 + 256],
                            xt8[t][:, :, mi * P:(mi + 1) * P],
                            wv8[:, :, :],
                            start=True, stop=True, perf_mode=DR,
                        )
                    nc.vector.tensor_copy(v8[h], vp)
                v8s.append(v8)

            def emit_s_mi(t, mi, e8):
                """S matmuls + exp for one m-chunk of teacher t."""
                p, j = mi // 2, mi % 2
                sp = mm.tile([P, N], F32, tag="mm", name=f"sp{t}{mi}")
                for nq in range(4):
                    nc.tensor.matmul(
                        sp[:, nq * 256:(nq + 1) * 256],
                        xt8[t][:, :, mi * P:(mi + 1) * P],
                        qg8[:, :, nq * 256:(nq + 1) * 256],
                        start=True, stop=True, perf_mode=DR,
                    )
                nc.scalar.activation(e8[p][:, j, :], sp, func=Exp,
                                     scale=SCALE, bias=c0_sb)

            def emit_zb(t, e8, recs):
                """Zb burst (3*colsum(E) broadcast) + reciprocal."""
                rec = rpool.tile([P, N], F32, tag="rec", name=f"rec{t}")
                for zt in range(2):
                    zb = zo.tile([P, 2, 512], F32, tag="zo",
                                 name=f"zb{t}{zt}")
                    for g in range(2):
                        nq = zt * 2 + g
                        for p in range(4):
                            nc.tensor.matmul(
                                zb[:, g, 0:256],
                                ones8,
                                e8[p][:, :, nq * 256:(nq + 1) * 256],
                                start=(p == 0), stop=(p == 3),
                                perf_mode=DR,
                            )
                    nc.vector.reciprocal(
                        rec[:, zt * 512:(zt + 1) * 512], zb[:, :, 0:256])
                recs.append(rec)

            def emit_o_chunk(t, co, ot, e8, v8, rec, tmp):
                """One (co, ot) O chunk: 8 matmuls + normalize mul."""
                op = zo.tile([P, 2, 512], F32, tag="zo", name=f"op{t}{co}{ot}")
                for g in range(2):
                    nq = ot * 2 + g
                    for p in range(4):
                        nc.tensor.matmul(
                            op[:, g, 0:256],
                            v8[p // 2][:, 2 * (p % 2):2 * (p % 2) + 2,
                                       co * P:(co + 1) * P],
                            e8[p][:, :, nq * 256:(nq + 1) * 256],
                            start=(p == 0), stop=(p == 3),
                            perf_mode=DR,
                        )
                nc.vector.tensor_mul(
                    tmp[:, ot * 512:(ot + 1) * 512], op[:, :, 0:256],
                    rec[:, ot * 512:(ot + 1) * 512],
                )

            e8s, v8s, recs = [], [], []
            for t in range(T):
                e8s.append([epool.tile([P, 2, N], F8, tag="e",
                                       name=f"e{t}{p}") for p in range(4)])

            # teacher 0: V + S (exp stream starts here)
            emit_v(0, v8s)
            for mi in range(8):
                emit_s_mi(0, mi, e8s[0])

            # teachers 1, 2: V + S with prior teacher's Zb/O interleaved
            # into the S mi-loop so PE work fills the Act-bound stalls
            for t in (1, 2):
                pt = t - 1
                emit_v(t, v8s)
                emit_s_mi(t, 0, e8s[t])
                emit_zb(pt, e8s[pt], recs)
                tmp = tpool.tile([P, N], F32, tag="tmp", name=f"tmp{pt}")
                emit_s_mi(t, 1, e8s[t])
                emit_o_chunk(pt, 0, 0, e8s[pt], v8s[pt], recs[pt], tmp)
                emit_s_mi(t, 2, e8s[t])
                emit_o_chunk(pt, 0, 1, e8s[pt], v8s[pt], recs[pt], tmp)
                emit_s_mi(t, 3, e8s[t])
                nc.gpsimd.tensor_add(acc[:, 0, :], acc[:, 0, :], tmp)
                emit_o_chunk(pt, 1, 0, e8s[pt], v8s[pt], recs[pt], tmp)
                emit_s_mi(t, 4, e8s[t])
                emit_o_chunk(pt, 1, 1, e8s[pt], v8s[pt], recs[pt], tmp)
                emit_s_mi(t, 5, e8s[t])
                nc.gpsimd.tensor_add(acc[:, 1, :], acc[:, 1, :], tmp)
                for mi in range(6, 8):
                    emit_s_mi(t, mi, e8s[t])

            # teacher 2 tail: Zb claimed first with pair-interleaved
            # matmuls (pairs 0-2 prefill during the exp stream), then O
            # chunks ordered to unblock normalize muls earliest; per-chunk
            # adds and out-DMAs spread over 4 DGE queues.
            rec2 = rpool.tile([P, N], F32, tag="rec", name="rec2")
            zbs = [zo.tile([P, 2, 512], F32, tag="zo", name=f"zb2{zt}")
                   for zt in range(2)]
            for p in range(4):
                for zt in range(2):
                    for g in range(2):
                        nq = zt * 2 + g
                        nc.tensor.matmul(
                            zbs[zt][:, g, 0:256], ones8,
                            e8s[2][p][:, :, nq * 256:(nq + 1) * 256],
                            start=(p == 0), stop=(p == 3), perf_mode=DR,
                        )
            for zt in range(2):
                nc.vector.reciprocal(
                    rec2[:, zt * 512:(zt + 1) * 512], zbs[zt][:, :, 0:256])
            recs.append(rec2)
            tmps = [tpool.tile([P, N], F32, tag="tmp", name=f"tmp2{co}")
                    for co in range(CC)]
            qeng = [nc.sync, nc.gpsimd, nc.scalar, nc.sync]
            for i, (co, ot) in enumerate([(0, 0), (1, 0), (0, 1), (1, 1)]):
                emit_o_chunk(2, co, ot, e8s[2], v8s[2], rec2, tmps[co])
                sl = slice(ot * 512, (ot + 1) * 512)
                nc.gpsimd.tensor_add(acc[:, co, sl], acc[:, co, sl],
                                     tmps[co][:, sl])
                qeng[i].dma_start(out=out_d[co][:, sl], in_=acc[:, co, sl])

    _split_multi_waits(nc)
    if not nc.is_finalized():
        nc.finalize()
    return nc


def _split_multi_waits(nc):
    """walrus can encode at most one sync-wait per instruction. Hoist every
    wait of a multi-wait instruction onto single-wait nops on the same
    engine, placed immediately before it in program order."""
    fixes = []
    for fn in nc.m.functions:
        for blk in fn.blocks:
            for inst in blk.instructions:
                si = getattr(inst, "sync_info", None)
                if (si is not None and si.on_wait and len(si.on_wait) > 1
                        and getattr(inst, "engine", None) is not None):
                    fixes.append((blk, inst))
    for blk, inst in fixes:
        si = inst.sync_info
        waits = list(si.on_wait)
        nops = []
        for w in waits:
            nop = nc.engines[inst.engine].nop(nofuse=True).ins
            nop.sync_info = mybir.SyncInfo(on_wait=[w], on_update=[])
            nops.append(nop)
        inst.sync_info = mybir.SyncInfo(on_wait=[], on_update=list(si.on_update))
        nop_names = {n.name for n in nops}
        for fn2 in nc.m.functions:
            for blk2 in fn2.blocks:
                blk2.instructions = [
                    i for i in blk2.instructions if i.name not in nop_names
                ]
        pos = next(i for i, x in enumerate(blk.instructions)
                   if x.name == inst.name)
        blk.instructions = (blk.instructions[:pos] + nops
                            + blk.instructions[pos:])


_NC = None


def _get_nc():
    global _NC
    if _NC is None:
        _NC = build_nc()
    return _NC


def make_in_maps(student_feat, t_feat0, t_feat1, t_feat2,
                 Wq, bq, Wk, bk, Wv, bv):
    # partition-major: [.., CC, P, ..] -> [.., P, CC, ..] so each tensor
    # loads in a single DMA
    xs = np.ascontiguousarray(
        student_feat.reshape(B, CC, P, N).transpose(0, 2, 1, 3),
        dtype=np.float32)
    xt = np.stack([t_feat0, t_feat1, t_feat2], axis=1).reshape(
        B, T, CC, P, N).transpose(0, 1, 3, 2, 4)
    # S^T = Xt^T (G Xs + h); G = Wk^T Wq (x16 for fp8 range, folded back
    # via the exp scale), h = 16 Wk^T bq.  bk cancels in the key softmax.
    Wq64 = np.asarray(Wq, np.float64)
    Wk64 = np.asarray(Wk, np.float64)
    gT = ((Wq64.T @ Wk64) * 16.0).reshape(CC, P, C).transpose(1, 0, 2)
    hq = ((Wk64.T @ np.asarray(bq, np.float64)) * 16.0).reshape(CC, P, 1)
    wvT = Wv.T.reshape(CC, P, C).transpose(1, 0, 2)
    hqc = np.ascontiguousarray(hq.transpose(1, 0, 2), dtype=np.float32)
    xsbv = np.ascontiguousarray(
        xs + np.asarray(bv, np.float32).reshape(CC, P, 1)
        .transpose(1, 0, 2)[None, :, :, :] * np.ones((B, 1, 1, 1), np.float32),
        dtype=np.float32)
    # host-side fp8 quantization (bit-identical to a device tensor_copy)
    xs8 = np.ascontiguousarray(np.asarray(xs, np.float32).astype(F8NP))
    xt8 = np.ascontiguousarray(np.asarray(xt, np.float32).astype(F8NP))
    g8 = np.ascontiguousarray(np.asarray(gT, np.float32).astype(F8NP))
    wv8 = np.ascontiguousarray(np.asarray(wvT, np.float32).astype(F8NP))
    return [
        {"xsbv": xsbv[b], "xs8": xs8[b], "xt8": xt8[b], "g8": g8, "wv8": wv8,
         "hq": hqc}
        for b in range(B)
    ]


def run(in_maps, trace=False):
    nc = _get_nc()
    return run_bass_kernel_spmd(nc, in_maps, core_ids=list(range(B)),
                                trace=trace)


def kernel(student_feat, t_feat0, t_feat1, t_feat2,
           Wq, bq, Wk, bk, Wv, bv):
    in_maps = make_in_maps(student_feat, t_feat0, t_feat1, t_feat2,
                           Wq, bq, Wk, bk, Wv, bv)
    res = run(in_maps, trace=False)
    out = np.stack([res.results[b]["out"].reshape(C, H, W) for b in range(B)])
    return out.astype(np.float32)
